# revision 31
# baseline (speedup 1.0000x reference)
"""Trainium2 Bass kernel for a 2-layer GCN over 2048 independent 25-node
KNN subgraphs (gnn_message_passing).

Strategy (mode "opt"):
  - Layer 1 is reassociated: relu(A @ (x @ W0)) == relu((A @ x) @ W0).
    The tiny sparse per-graph aggregation A @ x (25x25 @ 25x128 per
    graph, ~0.3 GFLOP) runs on host while packing inputs; the dominant
    dense transform (A x) @ W0 (3.4 GFLOP) runs on the PE. This removes
    the per-tile q matmul, its 512-col PSUM drain, and the 1.7 MB
    block-diagonal adjacency upload from the device entirely.
  - Host feeds (A x)^T feature-major, so each 128-node tile is a
    128x128 stationary operand (Fast Weight Load needs a non-fp32
    dtype and exactly 128 weight columns) streaming W0 (256 cols).
  - All matmul operands are bf16 (1 cy/col at any width); accumulation
    stays f32 in PSUM. rel-err budget is 2e-2, bf16 lands ~8e-3.
  - The layer-2 center aggregation (atc stationary h1-quarters, 5-col
    moving) writes straight into a single packed PSUM bank: fi-half 0
    in cols 0:256, half 1 in 256:512; the last tile has one real graph
    and writes 1 col, so 256 center slots fill the bank exactly.
  - The relu PSUM drain is split scalar/vector so neither drain engine
    limits the tensor pipe.
  - The ~6.5us framework preamble + DMA window leave the PE idle at a
    low DVFS state; junk warmup matmuls run in that shadow so the
    clock is ramped when real tiles arrive.
  - DMA: one big packed tensor (w0 | (Ax)^T) in 4 prioritized chunks
    on the sync HWDGE ring, one small aux tensor (atc | W1 | Wlin) on
    the scalar ring. ~2 MB total per core.
  - Data parallel over 8 cores: 256 graphs (52 tiles) per core.
"""

import os
import sys

import ml_dtypes
import numpy as np

for _p in ("/opt/trn_rl_repo", "/opt/trn_rl_repo/concourse"):
    if _p not in sys.path:
        sys.path.insert(0, _p)

import concourse.bass as bass
import concourse.tile as tile
from concourse import bacc, mybir
from concourse.bass_utils import run_bass_kernel_spmd

NCORES = 8
B = 2048            # graphs
K = 25              # nodes per graph
N = B * K           # 51200
GPC = B // NCORES   # 256 graphs per core
G = 5               # graphs packed per PE tile
P = G * K           # 125 real partitions used per tile
PT = 128            # padded tile width (FWL needs 128-col stationaries)
NT = (GPC + G - 1) // G   # 52 tiles per core (last tile: 1 real graph)
SLOTS = NT * G      # 260 graph slots in the host-side packing
CS = 256            # real center slots per core (= GPC), packed on device
CP = 5              # center-extraction columns per tile
F0 = 128            # input features
F1 = 256            # hidden features
XW = F1 + NT * PT   # w0 (256 cols) + (Ax)^T, packed in one dram tensor
AUXW = NT * CP + 2 * F1 + 2   # atc (260) | W1 (512) | Wlin (2)

_f32 = mybir.dt.float32
_bf16 = mybir.dt.bfloat16

_compiled = {}


def _build_nc_opt(warm):
    nc = bacc.Bacc("TRN2", target_bir_lowering=False, debug=False,
                   num_devices=NCORES)

    xw_d = nc.dram_tensor("xw", [F0, XW], _bf16, kind="ExternalInput")
    aux_d = nc.dram_tensor("aux", [128, AUXW], _bf16, kind="ExternalInput")
    out_d = nc.dram_tensor("out", [1, CS], _f32, kind="ExternalOutput")

    relu = mybir.ActivationFunctionType.Relu
    copy = mybir.ActivationFunctionType.Copy
    # chunk boundaries (in tiles) for the pipelined input DMAs
    xb = [0, 4, 12, 24, 40, NT]

    with tile.TileContext(nc) as tc:
        with (
            tc.tile_pool(name="const", bufs=1) as cpool,
            tc.tile_pool(name="h1p", bufs=3) as h1p,
            tc.tile_pool(name="outp", bufs=1) as outp,
            tc.tile_pool(name="ps_h1", bufs=3, space=bass.MemorySpace.PSUM) as ps_h1,
            tc.tile_pool(name="ps_p2", bufs=1, space=bass.MemorySpace.PSUM) as ps_p2,
            tc.tile_pool(name="ps_h3", bufs=2, space=bass.MemorySpace.PSUM) as ps_h3,
            tc.tile_pool(name="ps_o", bufs=1, space=bass.MemorySpace.PSUM) as ps_o,
        ):
            xw = cpool.tile([F0, XW], _bf16)
            aux = cpool.tile([128, AUXW], _bf16)
            junk = cpool.tile([128, 640], _bf16)

            # prioritized input DMAs: sync ring carries the bulk x-side in
            # first-needed-first order; the small aux tensor rides alone on
            # the scalar ring so it is never queued behind bulk.
            nc.scalar.dma_start(aux[:], aux_d[:])
            for c in range(5):
                lo, hi = xb[c], xb[c + 1]
                xlo = 0 if c == 0 else F1 + lo * PT
                # the last chunk rides the otherwise-idle scalar ring, so
                # the sync ring's supply stays ahead of loop consumption
                eng = nc.scalar if c == 4 else nc.sync
                eng.dma_start(xw[:, xlo:F1 + hi * PT],
                              xw_d[:, xlo:F1 + hi * PT])

            w0 = xw[:, 0:F1]
            # both center-aggregation halves packed into ONE psum bank:
            # fi-half 0 -> cols 0:256, fi-half 1 -> cols 256:512
            p2acc = ps_p2.tile([128, 512], _f32)

            # ---- engine warmup: junk work in the DMA-wait shadow so the
            # tensor/vector/scalar DVFS states are ramped when real tiles
            # arrive (all three pace the loop; all idle ~10us otherwise)
            if warm:
                nc.vector.memset(junk[:], 0.0)
                for w in range(warm):
                    nc.tensor.matmul(p2acc[:], junk[:, 0:128],
                                     junk[:, 128:640], start=True, stop=True)

            SG = 2   # PE tiles per super-tile
            for st in range(NT // SG):
                ts0 = SG * st

                h1_ps = ps_h1.tile([128, SG * F1], _f32)
                for j in range(SG):
                    nc.tensor.matmul(h1_ps[:, j * F1:(j + 1) * F1],
                                     xw[:, F1 + (ts0 + j) * PT:F1 + (ts0 + j + 1) * PT],
                                     w0, start=True, stop=True)
                h1_sb = h1p.tile([128, SG * F1], _bf16)
                if st % 2 == 0:
                    nc.vector.tensor_scalar_max(h1_sb[:], h1_ps[:], 0.0)
                else:
                    nc.scalar.activation(h1_sb[:], h1_ps[:], relu)

                for j in range(SG):
                    t, off = ts0 + j, j * F1
                    w = 1 if t == NT - 1 else CP   # last tile: 1 real graph
                    atc_t = aux[:, t * CP:t * CP + w]
                    nc.tensor.matmul(p2acc[:, t * G:t * G + w],
                                     h1_sb[:, off:off + 128], atc_t,
                                     start=True, stop=True)
                    nc.tensor.matmul(p2acc[:, 256 + t * G:256 + t * G + w],
                                     h1_sb[:, off + 128:off + 256], atc_t,
                                     start=True, stop=True)

            # ---- W1 transform over all centers (weight stationary) ----
            wb = NT * CP   # aux col where W1 starts
            p2s = cpool.tile([128, 2, CS], _bf16)
            nc.vector.tensor_copy(p2s[:, 0, :], p2acc[:, 0:256])
            nc.scalar.activation(p2s[:, 1, :], p2acc[:, 256:512], copy)

            h3_sb = cpool.tile([128, 2, CS], _bf16)
            for fo in range(2):
                h3_ps = ps_h3.tile([128, CS], _f32)
                for fi in range(2):
                    nc.tensor.matmul(h3_ps[:],
                                     aux[:, wb + fi * F1 + fo * 128:wb + fi * F1 + fo * 128 + 128],
                                     p2s[:, fi, :],
                                     start=(fi == 0), stop=(fi == 1))
                nc.scalar.activation(h3_sb[:, fo, :], h3_ps[:], relu)

            # ---- out = relu(h3).T @ Wlin ----
            out_ps = ps_o.tile([1, CS], _f32)
            for fo in range(2):
                nc.tensor.matmul(out_ps[:],
                                 aux[:, wb + 2 * F1 + fo:wb + 2 * F1 + fo + 1],
                                 h3_sb[:, fo, :],
                                 start=(fo == 0), stop=(fo == 1))
            out_sb = outp.tile([1, CS], _f32)
            nc.vector.tensor_copy(out_sb[:], out_ps[:])
            nc.sync.dma_start(out_d[:], out_sb[:])

    nc.compile()
    return nc


def _get_nc(mode):
    if mode not in _compiled:
        warm = int(os.environ.get("GCN_WARMUP", "4"))
        _compiled[mode] = _build_nc_opt(warm)
    return _compiled[mode]


def _host_prep_opt(x, edge_weight, W0, W1, Wlin, edge_index):
    bf = ml_dtypes.bfloat16
    src = edge_index[0].astype(np.int64)
    tgt = edge_index[1].astype(np.int64)
    b = src // K
    sl = src - b * K
    tl = tgt - (tgt // K) * K

    # dense raw adjacency per graph, indexed [b, t, s]
    idx = (b * K + tl) * K + sl
    Araw = np.bincount(idx, weights=edge_weight.astype(np.float64),
                       minlength=B * K * K).astype(np.float32).reshape(B, K, K)
    deg = Araw.sum(axis=2)                      # weighted in-degree [B, K]
    with np.errstate(divide="ignore"):
        dinv = np.where(deg > 0, 1.0 / np.sqrt(deg), 0.0).astype(np.float32)
    An = Araw * dinv[:, :, None] * dinv[:, None, :]   # [b, t, s]

    # layer-1 aggregation on host (tiny): ax[b] = An[b] @ x[b]  [B, K, F0]
    ax = np.matmul(An, x.reshape(B, K, F0))

    # center columns of the layer-2 adjacency: atc[b, s] = An[b, 0, s]
    # (aggregation onto the center node t=0), laid out [128, NT*CP]
    atc = np.zeros((NCORES, SLOTS, K), np.float32)
    atc[:, :GPC] = An[:, 0, :].reshape(NCORES, GPC, K)
    atc = atc.reshape(NCORES, NT, G, K)

    aux = np.zeros((NCORES, 128, AUXW), np.float32)
    ac = aux[:, :P, :NT * CP].reshape(NCORES, G, K, NT, G)
    for g in range(G):
        ac[:, g, :, :, g] = atc[:, :, g].transpose(0, 2, 1)
    wb = NT * CP
    aux[:, :, wb:wb + F1] = W1[None, 0:128, :]
    aux[:, :, wb + F1:wb + 2 * F1] = W1[None, 128:256, :]
    aux[:, :, wb + 2 * F1:] = Wlin.reshape(2, 128).T[None]
    aux = aux.astype(bf)

    # packed x tensor: [NC, F0, 256 (w0) + NT*PT ((Ax)^T, tile-padded)]
    axpad = np.zeros((NCORES, NT * P, F0), np.float32)
    axpad[:, :GPC * K] = ax.reshape(NCORES, GPC * K, F0)
    xw = np.zeros((NCORES, F0, XW), np.float32)
    xw[:, :, 0:F1] = W0[None, :, :]
    xw[:, :, F1:].reshape(NCORES, F0, NT, PT)[:, :, :, :P] = \
        axpad.reshape(NCORES, NT, P, F0).transpose(0, 3, 1, 2)
    xw = xw.astype(bf)

    in_maps = []
    for c in range(NCORES):
        in_maps.append({
            "xw": np.ascontiguousarray(xw[c]),
            "aux": np.ascontiguousarray(aux[c]),
        })
    return in_maps


def _run(inputs, mode="opt", trace=False):
    nc = _get_nc(mode)
    in_maps = _host_prep_opt(**inputs)
    res = run_bass_kernel_spmd(nc, in_maps, core_ids=list(range(NCORES)),
                               trace=trace)
    out = np.empty((B, 1), np.float32)
    for c in range(NCORES):
        out[c * GPC:(c + 1) * GPC, 0] = res.results[c]["out"][0, :GPC]
    return out, res


def kernel(**inputs):
    out, _ = _run(inputs, mode="opt", trace=False)
    return out


# revision 32
# speedup vs baseline: 1.0400x; 1.0400x over previous
"""Trainium2 Bass kernel for a 2-layer GCN over 2048 independent 25-node
KNN subgraphs (gnn_message_passing).

Strategy (mode "opt"):
  - Layer 1 is reassociated: relu(A @ (x @ W0)) == relu((A @ x) @ W0).
    The tiny sparse per-graph aggregation A @ x (25x25 @ 25x128 per
    graph, ~0.3 GFLOP) runs on host while packing inputs; the dominant
    dense transform (A x) @ W0 (3.4 GFLOP) runs on the PE. This removes
    the per-tile q matmul, its 512-col PSUM drain, and the 1.7 MB
    block-diagonal adjacency upload from the device entirely.
  - Host feeds (A x)^T feature-major, so each 128-node tile is a
    128x128 stationary operand (Fast Weight Load needs a non-fp32
    dtype and exactly 128 weight columns) streaming W0 (256 cols).
  - All matmul operands are bf16 (1 cy/col at any width); accumulation
    stays f32 in PSUM. rel-err budget is 2e-2, bf16 lands ~8e-3.
  - The layer-2 center aggregation (atc stationary h1-quarters, 5-col
    moving) writes straight into a single packed PSUM bank: fi-half 0
    in cols 0:256, half 1 in 256:512; the last tile has one real graph
    and writes 1 col, so 256 center slots fill the bank exactly.
  - The relu PSUM drain is split scalar/vector so neither drain engine
    limits the tensor pipe.
  - The ~6.5us framework preamble + DMA window leave the PE idle at a
    low DVFS state; junk warmup matmuls run in that shadow so the
    clock is ramped when real tiles arrive.
  - DMA: one big packed tensor (w0 | (Ax)^T) in 4 prioritized chunks
    on the sync HWDGE ring, one small aux tensor (atc | W1 | Wlin) on
    the scalar ring. ~2 MB total per core.
  - Data parallel over 8 cores: 256 graphs (52 tiles) per core.
"""

import os
import sys

import ml_dtypes
import numpy as np

for _p in ("/opt/trn_rl_repo", "/opt/trn_rl_repo/concourse"):
    if _p not in sys.path:
        sys.path.insert(0, _p)

import concourse.bass as bass
import concourse.tile as tile
from concourse import bacc, mybir
from concourse.bass_utils import run_bass_kernel_spmd

NCORES = 8
B = 2048            # graphs
K = 25              # nodes per graph
N = B * K           # 51200
GPC = B // NCORES   # 256 graphs per core
G = 5               # graphs packed per PE tile
P = G * K           # 125 real partitions used per tile
PT = 128            # padded tile width (FWL needs 128-col stationaries)
NT = (GPC + G - 1) // G   # 52 tiles per core (last tile: 1 real graph)
SLOTS = NT * G      # 260 graph slots in the host-side packing
CS = 256            # real center slots per core (= GPC), packed on device
CP = 5              # center-extraction columns per tile
F0 = 128            # input features
F1 = 256            # hidden features
XW = F1 + NT * PT   # w0 (256 cols) + (Ax)^T, packed in one dram tensor
AUXW = NT * CP + 2 * F1 + 2   # atc (260) | W1 (512) | Wlin (2)

_f32 = mybir.dt.float32
_bf16 = mybir.dt.bfloat16

_compiled = {}


def _build_nc_opt(warm):
    nc = bacc.Bacc("TRN2", target_bir_lowering=False, debug=False,
                   num_devices=NCORES)

    xw_d = nc.dram_tensor("xw", [F0, XW], _bf16, kind="ExternalInput")
    aux_d = nc.dram_tensor("aux", [128, AUXW], _bf16, kind="ExternalInput")
    out_d = nc.dram_tensor("out", [1, CS], _f32, kind="ExternalOutput")

    relu = mybir.ActivationFunctionType.Relu
    copy = mybir.ActivationFunctionType.Copy
    # chunk boundaries (in tiles) for the pipelined input DMAs
    xb = [0, 4, 12, 24, 40, NT]

    with tile.TileContext(nc) as tc:
        with (
            tc.tile_pool(name="const", bufs=1) as cpool,
            tc.tile_pool(name="h1p", bufs=3) as h1p,
            tc.tile_pool(name="outp", bufs=1) as outp,
            tc.tile_pool(name="ps_h1", bufs=3, space=bass.MemorySpace.PSUM) as ps_h1,
            tc.tile_pool(name="ps_p2", bufs=1, space=bass.MemorySpace.PSUM) as ps_p2,
            tc.tile_pool(name="ps_h3", bufs=2, space=bass.MemorySpace.PSUM) as ps_h3,
            tc.tile_pool(name="ps_o", bufs=1, space=bass.MemorySpace.PSUM) as ps_o,
        ):
            xw = cpool.tile([F0, XW], _bf16)
            aux = cpool.tile([128, AUXW], _bf16)
            junk = cpool.tile([128, 640], _bf16)

            # prioritized input DMAs: sync ring carries the bulk x-side in
            # first-needed-first order; the small aux tensor rides alone on
            # the scalar ring so it is never queued behind bulk.
            for c in range(5):
                lo, hi = xb[c], xb[c + 1]
                xlo = 0 if c == 0 else F1 + lo * PT
                nc.sync.dma_start(xw[:, xlo:F1 + hi * PT],
                                  xw_d[:, xlo:F1 + hi * PT])
                if c == 0:
                    nc.sync.dma_start(aux[:], aux_d[:])

            w0 = xw[:, 0:F1]
            # both center-aggregation halves packed into ONE psum bank:
            # fi-half 0 -> cols 0:256, fi-half 1 -> cols 256:512
            p2acc = ps_p2.tile([128, 512], _f32)

            # ---- engine warmup: junk work in the DMA-wait shadow so the
            # tensor/vector/scalar DVFS states are ramped when real tiles
            # arrive (all three pace the loop; all idle ~10us otherwise)
            if warm:
                nc.vector.memset(junk[:], 0.0)
                for w in range(warm):
                    nc.tensor.matmul(p2acc[:], junk[:, 0:128],
                                     junk[:, 128:640], start=True, stop=True)

            SG = 2   # PE tiles per super-tile
            for st in range(NT // SG):
                ts0 = SG * st

                h1_ps = ps_h1.tile([128, SG * F1], _f32)
                for j in range(SG):
                    nc.tensor.matmul(h1_ps[:, j * F1:(j + 1) * F1],
                                     xw[:, F1 + (ts0 + j) * PT:F1 + (ts0 + j + 1) * PT],
                                     w0, start=True, stop=True)
                h1_sb = h1p.tile([128, SG * F1], _bf16)
                if st % 2 == 0:
                    nc.vector.tensor_scalar_max(h1_sb[:], h1_ps[:], 0.0)
                else:
                    nc.scalar.activation(h1_sb[:], h1_ps[:], relu)

                for j in range(SG):
                    t, off = ts0 + j, j * F1
                    w = 1 if t == NT - 1 else CP   # last tile: 1 real graph
                    atc_t = aux[:, t * CP:t * CP + w]
                    nc.tensor.matmul(p2acc[:, t * G:t * G + w],
                                     h1_sb[:, off:off + 128], atc_t,
                                     start=True, stop=True)
                    nc.tensor.matmul(p2acc[:, 256 + t * G:256 + t * G + w],
                                     h1_sb[:, off + 128:off + 256], atc_t,
                                     start=True, stop=True)

            # ---- W1 transform over all centers (weight stationary) ----
            wb = NT * CP   # aux col where W1 starts
            p2s = cpool.tile([128, 2, CS], _bf16)
            nc.vector.tensor_copy(p2s[:, 0, :], p2acc[:, 0:256])
            nc.scalar.activation(p2s[:, 1, :], p2acc[:, 256:512], copy)

            h3_sb = cpool.tile([128, 2, CS], _bf16)
            for fo in range(2):
                h3_ps = ps_h3.tile([128, CS], _f32)
                for fi in range(2):
                    nc.tensor.matmul(h3_ps[:],
                                     aux[:, wb + fi * F1 + fo * 128:wb + fi * F1 + fo * 128 + 128],
                                     p2s[:, fi, :],
                                     start=(fi == 0), stop=(fi == 1))
                nc.scalar.activation(h3_sb[:, fo, :], h3_ps[:], relu)

            # ---- out = relu(h3).T @ Wlin ----
            out_ps = ps_o.tile([1, CS], _f32)
            for fo in range(2):
                nc.tensor.matmul(out_ps[:],
                                 aux[:, wb + 2 * F1 + fo:wb + 2 * F1 + fo + 1],
                                 h3_sb[:, fo, :],
                                 start=(fo == 0), stop=(fo == 1))
            out_sb = outp.tile([1, CS], _f32)
            nc.vector.tensor_copy(out_sb[:], out_ps[:])
            nc.sync.dma_start(out_d[:], out_sb[:])

    nc.compile()
    return nc


def _get_nc(mode):
    if mode not in _compiled:
        warm = int(os.environ.get("GCN_WARMUP", "4"))
        _compiled[mode] = _build_nc_opt(warm)
    return _compiled[mode]


def _host_prep_opt(x, edge_weight, W0, W1, Wlin, edge_index):
    bf = ml_dtypes.bfloat16
    src = edge_index[0].astype(np.int64)
    tgt = edge_index[1].astype(np.int64)
    b = src // K
    sl = src - b * K
    tl = tgt - (tgt // K) * K

    # dense raw adjacency per graph, indexed [b, t, s]
    idx = (b * K + tl) * K + sl
    Araw = np.bincount(idx, weights=edge_weight.astype(np.float64),
                       minlength=B * K * K).astype(np.float32).reshape(B, K, K)
    deg = Araw.sum(axis=2)                      # weighted in-degree [B, K]
    with np.errstate(divide="ignore"):
        dinv = np.where(deg > 0, 1.0 / np.sqrt(deg), 0.0).astype(np.float32)
    An = Araw * dinv[:, :, None] * dinv[:, None, :]   # [b, t, s]

    # layer-1 aggregation on host (tiny): ax[b] = An[b] @ x[b]  [B, K, F0]
    ax = np.matmul(An, x.reshape(B, K, F0))

    # center columns of the layer-2 adjacency: atc[b, s] = An[b, 0, s]
    # (aggregation onto the center node t=0), laid out [128, NT*CP]
    atc = np.zeros((NCORES, SLOTS, K), np.float32)
    atc[:, :GPC] = An[:, 0, :].reshape(NCORES, GPC, K)
    atc = atc.reshape(NCORES, NT, G, K)

    aux = np.zeros((NCORES, 128, AUXW), np.float32)
    ac = aux[:, :P, :NT * CP].reshape(NCORES, G, K, NT, G)
    for g in range(G):
        ac[:, g, :, :, g] = atc[:, :, g].transpose(0, 2, 1)
    wb = NT * CP
    aux[:, :, wb:wb + F1] = W1[None, 0:128, :]
    aux[:, :, wb + F1:wb + 2 * F1] = W1[None, 128:256, :]
    aux[:, :, wb + 2 * F1:] = Wlin.reshape(2, 128).T[None]
    aux = aux.astype(bf)

    # packed x tensor: [NC, F0, 256 (w0) + NT*PT ((Ax)^T, tile-padded)]
    axpad = np.zeros((NCORES, NT * P, F0), np.float32)
    axpad[:, :GPC * K] = ax.reshape(NCORES, GPC * K, F0)
    xw = np.zeros((NCORES, F0, XW), np.float32)
    xw[:, :, 0:F1] = W0[None, :, :]
    xw[:, :, F1:].reshape(NCORES, F0, NT, PT)[:, :, :, :P] = \
        axpad.reshape(NCORES, NT, P, F0).transpose(0, 3, 1, 2)
    xw = xw.astype(bf)

    in_maps = []
    for c in range(NCORES):
        in_maps.append({
            "xw": np.ascontiguousarray(xw[c]),
            "aux": np.ascontiguousarray(aux[c]),
        })
    return in_maps


def _run(inputs, mode="opt", trace=False):
    nc = _get_nc(mode)
    in_maps = _host_prep_opt(**inputs)
    res = run_bass_kernel_spmd(nc, in_maps, core_ids=list(range(NCORES)),
                               trace=trace)
    out = np.empty((B, 1), np.float32)
    for c in range(NCORES):
        out[c * GPC:(c + 1) * GPC, 0] = res.results[c]["out"][0, :GPC]
    return out, res


def kernel(**inputs):
    out, _ = _run(inputs, mode="opt", trace=False)
    return out
